# revision 43
# baseline (speedup 1.0000x reference)
"""Gemma3nTextDecoderLayer on 8 trn2 cores.

Sharding: token-sharded AltUp/norms/Laurel (256 tokens/core), head-sharded
attention (1 q-head/core, per-core kv-head weight slices), FF-sharded MLP
(1024 FF dims/core). Collectives: AG(x_normT) -> attention -> RS(o_partial)
-> AG(h_T) -> MLP -> RS(down_partial). Matmuls bf16, everything else fp32.
"""
import math
import os
import numpy as np
import ml_dtypes

import concourse.bass as bass
import concourse.mybir as mybir
import concourse.tile as tile
from concourse import bacc
from concourse.bass_utils import run_bass_kernel_spmd
from concourse.masks import make_identity

B, T, H = 2, 1024, 2048
NH, NKV, HD = 8, 2, 256
S = 4
FF = 8192
LR = 64
EPS = 1e-6
NCORE = 8
NTOK = B * T          # 2048 global tokens
TOK = NTOK // NCORE   # 256 per core
FFC = FF // NCORE     # 1024 per core
P = 128
HC = H // P           # 16 contraction chunks
RSQRT2 = 1.0 / math.sqrt(2.0)

fp32 = mybir.dt.float32
bf16 = mybir.dt.bfloat16
AF = mybir.ActivationFunctionType
ALU = mybir.AluOpType
BF = ml_dtypes.bfloat16

_NC_CACHE = None
_IDENT = None
_EPS_T = None


def _ln(nc, pool, x, out, D):
    """out = layernorm(x) over free dim D (weight=1). x fp32 SBUF [p, D]."""
    p = x.shape[0]
    nsub = max(1, D // 512)
    sub = D // nsub
    stats = pool.tile([p, nsub, 6], fp32, name="ln_stats")
    for i in range(nsub):
        nc.vector.bn_stats(out=stats[:, i, :], in_=x[:, i * sub:(i + 1) * sub])
    mv = pool.tile([p, 2], fp32, name="ln_mv")
    nc.vector.bn_aggr(out=mv[:], in_=stats[:])
    nc.scalar.activation(out=mv[:, 1:2], in_=mv[:, 1:2], func=AF.Sqrt,
                         bias=_EPS_T[:p, :])
    nc.vector.reciprocal(out=mv[:, 1:2], in_=mv[:, 1:2])
    nc.vector.tensor_scalar(out=out, in0=x, scalar1=mv[:, 0:1],
                            scalar2=mv[:, 1:2], op0=ALU.subtract, op1=ALU.mult)


def _transp_into(nc, ps_pool, dst, src, n=P):
    """dst[n,128] = src[128,n].T via PE; dst is an SBUF AP (any dtype)."""
    pt = ps_pool.tile([P, P], bf16, name="tp_ps")
    nc.tensor.transpose(pt[:n, :], src, _IDENT)
    nc.scalar.activation(out=dst, in_=pt[:n, :], func=AF.Copy)


def build_nc(debug=False, sim1=False):
    global _IDENT, _EPS_T
    nc = bacc.Bacc("TRN2", target_bir_lowering=False, debug=False,
                   num_devices=1 if sim1 else NCORE)
    nc._phase_marks = {}

    def _mark(name):
        nc._phase_marks[name] = nc.inst_id_counter if hasattr(nc, "inst_id_counter") else len(getattr(nc.main_func.blocks[0], "instructions", []))

    def din(name, shape, dt):
        return nc.dram_tensor(name, shape, dt, kind="ExternalInput").ap()

    h_in = din("h_in", [S, TOK, H], fp32)
    cosq = din("cosq", [NTOK, HD], fp32)
    sinq = din("sinq", [NTOK, HD], fp32)   # first half sign-flipped
    trimask = din("trimask", [P, P], fp32)
    # weights are pre-laid-out partition-major on the host so every DMA is
    # contiguous (strided (c p)->p c s gathers cost ~25us in descriptors)
    wqkT = din("wqkT", [P, HC, 2 * HD], bf16)  # [:256]=q head, [256:]=k head
    wvT = din("wvT", [P, HC, HD], bf16)
    woT = din("woT", [P, HC, H], bf16)     # full wo^T (o-proj is token-local)
    gateT = din("gateT", [P, 8, HC, P], bf16)
    upT = din("upT", [P, 8, HC, P], bf16)
    downT = din("downT", [P, 8, H], bf16)
    llT = din("llT", [P, HC, LR], bf16)
    lrT = din("lrT", [LR, H], bf16)
    routerT = din("routerT", [P, HC, S], bf16)
    predT = din("predT", [S, S * S], bf16)
    corrT = din("corrT", [S, S], bf16)
    coscale = din("coscale", [P, H], fp32)

    out_d = nc.dram_tensor("out", [S, TOK, H], fp32, kind="ExternalOutput").ap()
    dbg = {}
    if debug:
        for nm, shp, dt in [("dbg_xnorm", [TOK, H], fp32),
                            ("dbg_laurel", [TOK, H], fp32),
                            ("dbg_m", [S, TOK], bf16),
                            ("dbg_qro", [16, P, HD], bf16),
                            ("dbg_kro", [16, P, HD], bf16),
                            ("dbg_vn", [16, P, HD], bf16),
                            ("dbg_attn", [TOK, H], fp32),
                            ("dbg_al", [TOK, H], fp32),
                            ("dbg_hmlp", [TOK, H], fp32),
                            ("dbg_ffw", [TOK, H], fp32),
                            ("dbg_act", [TOK, H], fp32),
                            ("dbg_cc", [TOK, S], fp32),
                            ("dbg_sc", [16, P, 1024], fp32),
                            ("dbg_pbf", [16, P, 1024], bf16),
                            ("dbg_av", [16, P, 256], bf16),]:
            dbg[nm] = nc.dram_tensor(nm, shp, dt, kind="ExternalOutput").ap()

    pred_d = nc.dram_tensor("pred_d", [S, TOK, H], fp32).ap()
    al_d = nc.dram_tensor("al_d", [TOK, H], fp32).ap()
    # AG1 chunked by local token half (fires as soon as each 128-token
    # block's x_normT is stored, overlapping the wire with laurel/altup)
    ag1_in = [nc.dram_tensor(f"ag1_in{t}", [P, HC, P], bf16).ap()
              for t in range(2)]
    ag1_out = [nc.dram_tensor(f"ag1_out{t}", [NCORE, P, HC, P], bf16,
                              addr_space="Shared").ap() for t in range(2)]
    # attention output exchanged via AllToAll of per-head AV (1MB vs the 8MB
    # o-partial ReduceScatter); o-proj then runs token-locally in phase C
    a2a_in = nc.dram_tensor("a2a_in", [NTOK, HD], bf16).ap()
    a2a_out = nc.dram_tensor("a2a_out", [NTOK, HD], bf16).ap()
    ag2_in = nc.dram_tensor("ag2_in", [P, HC, TOK], bf16).ap()
    ag2_out = nc.dram_tensor("ag2_out", [NCORE, P, HC, TOK], bf16,
                             addr_space="Shared").ap()
    rs2_in = [nc.dram_tensor(f"rs2_in{q}", [NTOK, H // 2], bf16).ap()
              for q in range(2)]
    rs2_out = [nc.dram_tensor(f"rs2_out{q}", [TOK, H // 2], bf16).ap()
               for q in range(2)]

    RG = [list(range(NCORE))]

    with tile.TileContext(nc) as tc:
        with (
            tc.tile_pool(name="const", bufs=1) as const,
            tc.tile_pool(name="tpps", bufs=2, space="PSUM") as tpps,
        ):
            ident = const.tile([P, P], bf16)
            make_identity(nc, ident)
            _IDENT = ident[:]
            eps_t = const.tile([P, 1], fp32)
            nc.gpsimd.memset(eps_t[:], EPS)
            _EPS_T = eps_t
            routerT_sb = const.tile([P, HC, S], bf16)
            nc.sync.dma_start(routerT_sb[:], routerT[:])
            predT_sb = const.tile([S, S * S], bf16)
            nc.sync.dma_start(predT_sb[:], predT[:])
            corrT_sb = const.tile([S, S], bf16)
            nc.sync.dma_start(corrT_sb[:], corrT[:])

            # ---------------- Phases A-C need pred0/laurel kept ------------
            with tc.tile_pool(name="keep", bufs=1) as keep:
                pred0_keep = keep.tile([P, 2, H], fp32)
                laurel_keep = keep.tile([P, 2, H], fp32)

                # ---- Phase A: altup predict + x_norm + laurel ----
                with (
                    tc.tile_pool(name="pa", bufs=2) as pa,
                    tc.tile_pool(name="pa2", bufs=1) as pa2,
                    tc.tile_pool(name="pa_ps", bufs=1, space="PSUM") as pa_ps,
                    tc.tile_pool(name="pa_mps", bufs=2, space="PSUM") as pa_mps,
                ):
                    # pass 1: per 128-token block, minimal chain to x_normT so
                    # AG1 can fire; altup j=1..3 + laurel run later under the
                    # AG1 wire. All gpsimd DMAs precede the first collective
                    # trigger (gpsimd is strict FIFO).
                    actx = []
                    for tt in range(2):
                        tsl = slice(tt * P, (tt + 1) * P)
                        hf0 = pa.tile([P, H], fp32, name="hf0", bufs=1)
                        nc.sync.dma_start(hf0[:], h_in[0, tsl, :])
                        hb = pa.tile([P, S, H], bf16, name="hb")
                        for s in range(S):
                            nc.gpsimd.dma_start(hb[:, s, :], h_in[s, tsl, :])
                        # router LN of h0 -> transpose -> m -> tanh
                        r_bf = pa.tile([P, H], bf16, name="r_bf", bufs=1)
                        _ln(nc, pa, hf0[:], r_bf[:], H)
                        rT = pa.tile([P, HC, P], bf16, name="rT", bufs=1)
                        for hc in range(HC):
                            _transp_into(nc, tpps, rT[:, hc, :],
                                         r_bf[:, hc * P:(hc + 1) * P])
                        m_ps = pa_mps.tile([S, P], fp32, space="PSUM", name="m_ps")
                        for hc in range(HC):
                            nc.tensor.matmul(m_ps[:], routerT_sb[:, hc, :],
                                             rT[:, hc, :],
                                             start=(hc == 0), stop=(hc == HC - 1))
                        m_sb = pa.tile([S, P], bf16, name="m_sb")
                        nc.scalar.activation(out=m_sb[:], in_=m_ps[:], func=AF.Tanh,
                                             scale=1.0 / H)
                        c_ps = pa_ps.tile([P, S * S], fp32, space="PSUM", name="c_ps")
                        nc.tensor.matmul(c_ps[:], m_sb[:], predT_sb[:],
                                         start=True, stop=True)
                        coef = pa.tile([P, S * S], fp32, name="coef")
                        nc.scalar.activation(out=coef[:], in_=c_ps[:], func=AF.Copy)

                        # predictions: j=0 first so x_norm -> AG1 fires ASAP;
                        # j=1..3 and laurel run under the AG1 wire.
                        def predict(j, tt=tt, tsl=tsl, hb=hb, coef=coef, hf0=hf0):
                            a01 = pa.tile([P, H], bf16, name="a01")
                            a23 = pa.tile([P, H], bf16, name="a23")
                            for pair, dst in ((0, a01), (1, a23)):
                                tma = pa.tile([P, H], bf16, name="tma")
                                tmb = pa.tile([P, H], bf16, name="tmb")
                                for s, tm in ((2 * pair, tma), (2 * pair + 1, tmb)):
                                    nc.vector.tensor_scalar_mul(
                                        out=tm[:], in0=hb[:, s, :],
                                        scalar1=coef[:, j * S + s:j * S + s + 1])
                                nc.vector.tensor_add(out=dst[:], in0=tma[:],
                                                     in1=tmb[:])
                            d32 = pa.tile([P, H], fp32, name="d32")
                            nc.vector.tensor_add(out=d32[:], in0=a01[:], in1=a23[:])
                            if j == 0:
                                pj = pred0_keep[:, tt, :]
                                hfj = hf0
                            else:
                                pjt = pa2.tile([P, H], fp32, name="pjt")
                                pj = pjt[:]
                                hfj = pa2.tile([P, H], fp32, name="hfj")
                                nc.sync.dma_start(hfj[:], h_in[j, tsl, :])
                            nc.vector.tensor_add(out=pj, in0=hfj[:], in1=d32[:])
                            nc.sync.dma_start(pred_d[j, tsl, :], pj)

                        predict(0)
                        # x_norm / ag1
                        xn = pa.tile([P, H], fp32, name="xn")
                        _ln(nc, pa, pred0_keep[:, tt, :], xn[:], H)
                        xn_bf = pa.tile([P, H], bf16, name="xn_bf")
                        nc.scalar.activation(out=xn_bf[:], in_=xn[:], func=AF.Copy)
                        xnT = pa.tile([P, HC, P], bf16, name="xnT")
                        for hc in range(HC):
                            _transp_into(nc, tpps, xnT[:, hc, :],
                                         xn_bf[:, hc * P:(hc + 1) * P])
                        nc.sync.dma_start(ag1_in[tt][:], xnT[:])
                        actx.append((tsl, predict, xn, xnT, m_sb))

                    for tt in range(2):
                        if sim1:
                            for r_ in range(NCORE):
                                nc.sync.dma_start(ag1_out[tt][r_], ag1_in[tt][:])
                        else:
                            nc.gpsimd.collective_compute(
                                "AllGather", ALU.bypass, replica_groups=RG,
                                ins=[ag1_in[tt].opt()],
                                outs=[ag1_out[tt].opt()])

                    llT_sb = pa.tile([P, HC, LR], bf16, bufs=1)
                    nc.sync.dma_start(llT_sb[:], llT[:])
                    lrT_sb = pa.tile([LR, H], bf16, bufs=1)
                    nc.sync.dma_start(lrT_sb[:], lrT[:])
                    # pass 2: altup predictions j=1..3 + laurel (AG1 filler)
                    for tt in range(2):
                        tsl, predict, xn, xnT, m_sb = actx[tt]
                        for j in range(1, S):
                            predict(j)
                        l1_ps = pa_ps.tile([P, LR], fp32, space="PSUM", name="l1_ps")
                        for hc in range(HC):
                            nc.tensor.matmul(l1_ps[:], xnT[:, hc, :],
                                             llT_sb[:, hc, :],
                                             start=(hc == 0), stop=(hc == HC - 1))
                        l1_bf = pa.tile([P, LR], bf16, name="l1_bf")
                        nc.scalar.activation(out=l1_bf[:], in_=l1_ps[:], func=AF.Copy)
                        l1T = pa.tile([LR, P], bf16, name="l1T")
                        _transp_into(nc, tpps, l1T[:], l1_bf[:], n=LR)
                        l2 = pa.tile([P, H], fp32, name="l2", bufs=1)
                        for n4 in range(4):
                            nsl = slice(n4 * 512, (n4 + 1) * 512)
                            l2_ps = pa_ps.tile([P, 512], fp32, space="PSUM",
                                               name="l2_ps")
                            nc.tensor.matmul(l2_ps[:], l1T[:], lrT_sb[:, nsl],
                                             start=True, stop=True)
                            nc.scalar.activation(out=l2[:, nsl], in_=l2_ps[:],
                                                 func=AF.Copy)
                        l2ln = pa.tile([P, H], fp32, name="l2ln", bufs=1)
                        _ln(nc, pa, l2[:], l2ln[:], H)
                        nc.vector.tensor_add(out=laurel_keep[:, tt, :], in0=xn[:],
                                             in1=l2ln[:])
                        if debug:
                            nc.sync.dma_start(dbg["dbg_xnorm"][tsl, :], xn[:])
                            nc.sync.dma_start(dbg["dbg_laurel"][tsl, :],
                                              laurel_keep[:, tt, :])
                            nc.sync.dma_start(dbg["dbg_m"][:, tsl], m_sb[:])

                # ---- Phase B: attention ----
                with (
                    tc.tile_pool(name="pb_w", bufs=1) as pb_w,
                    tc.tile_pool(name="pb_qkv", bufs=1) as pb_qkv,
                    tc.tile_pool(name="pb_x", bufs=3) as pb_x,
                    tc.tile_pool(name="pb_t", bufs=2) as pb_t,
                    tc.tile_pool(name="pb_ps", bufs=1, space="PSUM") as pb_ps,
                    tc.tile_pool(name="pb_sc", bufs=1, space="PSUM") as pb_sc,
                ):
                    wqkT_sb = pb_w.tile([P, HC, 2 * HD], bf16)
                    nc.sync.dma_start(wqkT_sb[:], wqkT[:])
                    wvT_sb = pb_w.tile([P, HC, HD], bf16)
                    nc.sync.dma_start(wvT_sb[:], wvT[:])
                    trim_sb = pb_w.tile([P, P], fp32)
                    nc.sync.dma_start(trim_sb[:], trimask[:])
                    q_fm = pb_qkv.tile([P, 2, 16, P], bf16)
                    k_fm = pb_qkv.tile([P, 2, 16, P], bf16)
                    v_sb = pb_qkv.tile([P, 16, HD], bf16)

                    # evens first: blocks with tb%2==0 only need the first
                    # AG1 chunk, so qkv starts one collective earlier
                    for tb in [0, 2, 4, 6, 8, 10, 12, 14, 1, 3, 5, 7, 9, 11, 13, 15]:
                        xfm = pb_x.tile([P, HC, P], bf16, name="xfm")
                        nc.sync.dma_start(xfm[:], ag1_out[tb % 2][tb // 2])
                        qk_ps = pb_ps.tile([P, 512], fp32, space="PSUM",
                                           name="qk_ps")
                        v_ps = pb_ps.tile([P, 512], fp32, space="PSUM", name="v_ps")
                        for hc in range(HC):
                            nc.tensor.matmul(qk_ps[:], xfm[:, hc, :],
                                             wqkT_sb[:, hc, :],
                                             start=(hc == 0), stop=(hc == HC - 1))
                            nc.tensor.matmul(v_ps[:, :HD], xfm[:, hc, :],
                                             wvT_sb[:, hc, :],
                                             start=(hc == 0), stop=(hc == HC - 1))
                        cs = pb_x.tile([P, 2, HD], fp32, name="cs")
                        nc.sync.dma_start(cs[:, 0, :], cosq[tb * P:(tb + 1) * P, :])
                        nc.sync.dma_start(cs[:, 1, :], sinq[tb * P:(tb + 1) * P, :])
                        for which, psrc in (("q", qk_ps[:, :HD]),
                                            ("k", qk_ps[:, HD:]),
                                            ("v", v_ps[:, :HD])):
                            xsb = pb_t.tile([P, HD], fp32, name=f"{which}_sb")
                            nc.scalar.activation(out=xsb[:], in_=psrc, func=AF.Copy)
                            nrm = pb_t.tile([P, HD], fp32, name=f"{which}_n")
                            _ln(nc, pb_t, xsb[:], nrm[:], HD)
                            if which == "v":
                                nc.vector.tensor_copy(out=v_sb[:, tb, :],
                                                      in_=nrm[:])
                                if debug:
                                    nc.sync.dma_start(dbg["dbg_vn"][tb, :, :],
                                                      v_sb[:, tb, :])
                                continue
                            t1 = pb_t.tile([P, HD], fp32, name=f"{which}_t1")
                            nc.vector.tensor_mul(out=t1[:], in0=nrm[:],
                                                 in1=cs[:, 0, :])
                            t2 = pb_t.tile([P, HD], fp32, name=f"{which}_t2")
                            hh = HD // 2
                            nc.vector.tensor_mul(out=t2[:, :hh], in0=nrm[:, hh:],
                                                 in1=cs[:, 1, :hh])
                            nc.vector.tensor_mul(out=t2[:, hh:], in0=nrm[:, :hh],
                                                 in1=cs[:, 1, hh:])
                            ro = pb_t.tile([P, HD], bf16, name=f"{which}_ro")
                            nc.vector.tensor_add(out=ro[:], in0=t1[:], in1=t2[:])
                            if debug:
                                nm = "dbg_qro" if which == "q" else "dbg_kro"
                                nc.sync.dma_start(dbg[nm][tb, :, :], ro[:])
                            dst = q_fm if which == "q" else k_fm
                            for h2 in range(2):
                                _transp_into(nc, tpps, dst[:, h2, tb, :],
                                             ro[:, h2 * P:(h2 + 1) * P])

                    for b in range(2):
                        for qi in range(8):
                            tbq = b * 8 + qi
                            W = (qi + 1) * P
                            nseg = (W + 511) // 512
                            segs = [pb_sc.tile([P, 512], fp32, space="PSUM",
                                               name=f"sc{g}")
                                    for g in range(nseg)]
                            for g in range(nseg):
                                w_ = min(512, W - g * 512)
                                nblk = w_ // P
                                kb0 = b * 8 + g * 4
                                for h2 in range(2):
                                    nc.tensor.matmul(
                                        segs[g][:, :w_], q_fm[:, h2, tbq, :],
                                        k_fm[:, h2, kb0:kb0 + nblk, :].rearrange(
                                            "p a b -> p (a b)"),
                                        start=(h2 == 0), stop=(h2 == 1))
                            sc = pb_t.tile([P, 1024], fp32, name="sc_sb")
                            for g in range(nseg):
                                w_ = min(512, W - g * 512)
                                nc.scalar.activation(
                                    out=sc[:, g * 512:g * 512 + w_],
                                    in_=segs[g][:, :w_], func=AF.Copy)
                            nc.vector.tensor_add(out=sc[:, W - P:W],
                                                 in0=sc[:, W - P:W], in1=trim_sb[:])
                            if debug:
                                nc.sync.dma_start(dbg["dbg_sc"][tbq, :, :W],
                                                  sc[:, :W])
                            mx = pb_t.tile([P, 1], fp32, name="mx")
                            nc.vector.reduce_max(out=mx[:], in_=sc[:, :W],
                                                 axis=mybir.AxisListType.X)
                            nc.vector.tensor_scalar_mul(out=mx[:], in0=mx[:],
                                                        scalar1=-1.0)
                            rsum = pb_t.tile([P, 1], fp32, name="rsum")
                            pexp = pb_t.tile([P, 1024], fp32, name="pexp")
                            nc.scalar.activation(out=pexp[:, :W], in_=sc[:, :W],
                                                 func=AF.Exp, bias=mx[:],
                                                 accum_out=rsum[:])
                            nc.vector.reciprocal(out=rsum[:], in_=rsum[:])
                            pbf = pb_t.tile([P, 1024], bf16, name="pbf")
                            nc.vector.tensor_scalar_mul(out=pbf[:, :W],
                                                        in0=pexp[:, :W],
                                                        scalar1=rsum[:])
                            if debug:
                                nc.sync.dma_start(dbg["dbg_pbf"][tbq, :, :W],
                                                  pbf[:, :W])
                            ptall = pb_t.tile([P, 8, P], bf16, name="ptall")
                            for kc in range(qi + 1):
                                _transp_into(nc, tpps, ptall[:, kc, :],
                                             pbf[:, kc * P:(kc + 1) * P])
                            # av2[tq, hd] = P @ V, p-transposed stationary
                            av_ps = pb_ps.tile([P, 256], fp32, space="PSUM",
                                               name="av_ps")
                            for kc in range(qi + 1):
                                nc.tensor.matmul(
                                    av_ps[:], ptall[:, kc, :],
                                    v_sb[:, b * 8 + kc, :],
                                    start=(kc == 0), stop=(kc == qi))
                            av2_bf = pb_t.tile([P, 256], bf16, name="av2_bf")
                            nc.scalar.activation(out=av2_bf[:], in_=av_ps[:],
                                                 func=AF.Copy)
                            if debug:
                                nc.sync.dma_start(dbg["dbg_av"][tbq, :, :],
                                                  av2_bf[:])
                            nc.sync.dma_start(a2a_in[tbq * P:(tbq + 1) * P, :],
                                              av2_bf[:])

                if sim1:
                    nc.sync.dma_start(a2a_out[:], a2a_in[:])
                else:
                    nc.gpsimd.collective_compute(
                        "AllToAll", ALU.bypass, replica_groups=RG,
                        ins=[a2a_in.opt()], outs=[a2a_out.opt()])

                # ---- Phase C: local o-proj + mid (token-local) ----
                with (
                    tc.tile_pool(name="pc", bufs=2) as pc,
                    tc.tile_pool(name="pc_ps", bufs=2, space="PSUM") as pc_ps,
                ):
                    woT_sb = pc.tile([P, HC, H], bf16, bufs=1)
                    nc.sync.dma_start(woT_sb[:], woT[:])
                    for tt in range(2):
                        tsl = slice(tt * P, (tt + 1) * P)
                        av_in = pc.tile([P, 8, HD], bf16, name="av_in")
                        for j in range(8):
                            r0 = j * TOK + tt * P
                            nc.sync.dma_start(av_in[:, j, :],
                                              a2a_out[r0:r0 + P, :])
                        avT = pc.tile([P, HC, P], bf16, name="avT")
                        for hc in range(HC):
                            _transp_into(
                                nc, tpps, avT[:, hc, :],
                                av_in[:, hc // 2,
                                      (hc % 2) * P:(hc % 2) * P + P])
                        o32 = pc.tile([P, H], fp32, name="o32")
                        for n4 in range(4):
                            nsl = slice(n4 * 512, (n4 + 1) * 512)
                            o_ps = pc_ps.tile([P, 512], fp32, space="PSUM",
                                              name="o_ps")
                            for hc in range(HC):
                                nc.tensor.matmul(o_ps[:], avT[:, hc, :],
                                                 woT_sb[:, hc, nsl],
                                                 start=(hc == 0),
                                                 stop=(hc == HC - 1))
                            nc.scalar.activation(out=o32[:, nsl], in_=o_ps[:],
                                                 func=AF.Copy)
                        oln = pc.tile([P, H], fp32, name="oln")
                        _ln(nc, pc, o32[:], oln[:], H)
                        gated = pc.tile([P, H], fp32, name="gated", bufs=1)
                        nc.vector.tensor_add(out=gated[:],
                                             in0=pred0_keep[:, tt, :], in1=oln[:])
                        nc.vector.tensor_add(out=gated[:], in0=gated[:],
                                             in1=laurel_keep[:, tt, :])
                        al = pc.tile([P, H], fp32, name="al", bufs=1)
                        nc.vector.tensor_scalar_mul(out=al[:], in0=gated[:],
                                                    scalar1=RSQRT2)
                        nc.sync.dma_start(al_d[tsl, :], al[:])
                        hmlp = pc.tile([P, H], fp32, name="hmlp")
                        _ln(nc, pc, al[:], hmlp[:], H)
                        if debug:
                            nc.sync.dma_start(dbg["dbg_attn"][tsl, :], o32[:])
                            nc.sync.dma_start(dbg["dbg_al"][tsl, :], al[:])
                            nc.sync.dma_start(dbg["dbg_hmlp"][tsl, :], hmlp[:])
                        h_bf = pc.tile([P, H], bf16, name="h_bf")
                        nc.scalar.activation(out=h_bf[:], in_=hmlp[:], func=AF.Copy)
                        hT = pc.tile([P, HC, P], bf16, name="hT")
                        for hc in range(HC):
                            _transp_into(nc, tpps, hT[:, hc, :],
                                         h_bf[:, hc * P:(hc + 1) * P])
                        nc.sync.dma_start(ag2_in[:, :, tsl], hT[:])

            if sim1:
                for r_ in range(NCORE):
                    nc.sync.dma_start(ag2_out[r_], ag2_in[:])
            else:
                nc.gpsimd.collective_compute(
                    "AllGather", ALU.bypass, replica_groups=RG,
                    ins=[ag2_in.opt()], outs=[ag2_out.opt()])

            # ---- Phase D: MLP ----
            with (
                tc.tile_pool(name="pd_h", bufs=1) as pd_h,
                tc.tile_pool(name="pd_w", bufs=2) as pd_w,
                tc.tile_pool(name="pd_t", bufs=2) as pd_t,
                tc.tile_pool(name="pd_ps", bufs=2, space="PSUM") as pd_ps,
            ):
                h_fm = pd_h.tile([P, HC, NTOK], bf16)
                for r in range(NCORE):
                    nc.sync.dma_start(h_fm[:, :, r * TOK:(r + 1) * TOK],
                                      ag2_out[r])
                act_fm = pd_h.tile([P, 8, NTOK], bf16)
                downT_sb = pd_h.tile([P, 8, H], bf16)
                nc.sync.dma_start(downT_sb[:], downT[:])
                for mc in range(8):
                    gT = pd_w.tile([P, HC, P], bf16, name="gT")
                    uT = pd_w.tile([P, HC, P], bf16, name="uT")
                    nc.sync.dma_start(gT[:], gateT[:, mc])
                    nc.sync.dma_start(uT[:], upT[:, mc])
                    for ntc in range(4):
                        nsl = slice(ntc * 512, (ntc + 1) * 512)
                        g_ps = pd_ps.tile([P, 512], fp32, space="PSUM", name="g_ps")
                        u_ps = pd_ps.tile([P, 512], fp32, space="PSUM", name="u_ps")
                        for hc in range(HC):
                            hf = h_fm[:, hc, nsl]
                            nc.tensor.matmul(g_ps[:], gT[:, hc, :], hf,
                                             start=(hc == 0), stop=(hc == HC - 1))
                            nc.tensor.matmul(u_ps[:], uT[:, hc, :], hf,
                                             start=(hc == 0), stop=(hc == HC - 1))
                        gel = pd_t.tile([P, 512], bf16, name="gel")
                        nc.scalar.activation(out=gel[:], in_=g_ps[:],
                                             func=AF.Gelu_apprx_tanh)
                        ub = pd_t.tile([P, 512], bf16, name="ub")
                        nc.vector.tensor_copy(out=ub[:], in_=u_ps[:])
                        nc.vector.tensor_mul(out=act_fm[:, mc, nsl], in0=gel[:],
                                             in1=ub[:])
                # down-proj in H-halves so the first RS2 chunk's wire
                # overlaps the second half's compute
                for half in range(2):
                    for tc_ in range(16):
                        d_sb = pd_t.tile([P, H // 2], bf16, name="d_sb")
                        for n2 in range(2):
                            nsl = slice(half * 1024 + n2 * 512,
                                        half * 1024 + (n2 + 1) * 512)
                            d_ps = pd_ps.tile([P, 512], fp32, space="PSUM",
                                              name="d_ps")
                            for fc in range(8):
                                nc.tensor.matmul(
                                    d_ps[:], act_fm[:, fc, tc_ * P:(tc_ + 1) * P],
                                    downT_sb[:, fc, nsl],
                                    start=(fc == 0), stop=(fc == 7))
                            nc.scalar.activation(
                                out=d_sb[:, n2 * 512:(n2 + 1) * 512],
                                in_=d_ps[:], func=AF.Copy)
                        nc.sync.dma_start(
                            rs2_in[half][tc_ * P:(tc_ + 1) * P, :], d_sb[:])
                    if sim1:
                        nc.sync.dma_start(rs2_out[half][:], rs2_in[half][:TOK, :])
                    else:
                        nc.gpsimd.collective_compute(
                            "ReduceScatter", ALU.add, replica_groups=RG,
                            ins=[rs2_in[half].opt()], outs=[rs2_out[half].opt()])

            # ---- Phase E: altup correct ----
            with (
                tc.tile_pool(name="pe", bufs=2) as pe,
                tc.tile_pool(name="pe_ps", bufs=2, space="PSUM") as pe_ps,
            ):
                cosc_sb = pe.tile([P, H], fp32, bufs=1, name="cosc_sb")
                nc.sync.dma_start(cosc_sb[:], coscale[:])
                for tt in range(2):
                    tsl = slice(tt * P, (tt + 1) * P)
                    fbf = pe.tile([P, H], bf16, name="fbf")
                    for half in range(2):
                        nc.sync.dma_start(
                            fbf[:, half * 1024:(half + 1) * 1024],
                            rs2_out[half][tsl, :])
                    fln = pe.tile([P, H], fp32, name="fln")
                    _ln(nc, pe, fbf[:], fln[:], H)
                    alr = pe.tile([P, H], fp32, name="alr")
                    nc.sync.dma_start(alr[:], al_d[tsl, :])
                    p0 = pe.tile([P, H], fp32, name="p0")
                    nc.sync.dma_start(p0[:], pred_d[0, tsl, :])
                    act32 = pe.tile([P, H], fp32, name="act32")
                    nc.vector.tensor_add(out=act32[:], in0=alr[:], in1=fln[:])
                    r2 = pe.tile([P, H], bf16, name="r2")
                    _ln(nc, pe, act32[:], r2[:], H)
                    r2T = pe.tile([P, HC, P], bf16, name="r2T")
                    for hc in range(HC):
                        _transp_into(nc, tpps, r2T[:, hc, :],
                                     r2[:, hc * P:(hc + 1) * P])
                    m2_ps = pe_ps.tile([S, P], fp32, space="PSUM", name="m2_ps")
                    for hc in range(HC):
                        nc.tensor.matmul(m2_ps[:], routerT_sb[:, hc, :],
                                         r2T[:, hc, :],
                                         start=(hc == 0), stop=(hc == HC - 1))
                    m2_sb = pe.tile([S, P], bf16, name="m2_sb")
                    nc.scalar.activation(out=m2_sb[:], in_=m2_ps[:], func=AF.Tanh,
                                         scale=1.0 / H)
                    cc_ps = pe_ps.tile([P, S], fp32, space="PSUM", name="cc_ps")
                    nc.tensor.matmul(cc_ps[:], m2_sb[:], corrT_sb[:],
                                     start=True, stop=True)
                    cc = pe.tile([P, S], fp32, name="cc")
                    nc.scalar.activation(out=cc[:], in_=cc_ps[:], func=AF.Copy)
                    nc.vector.tensor_scalar_add(out=cc[:], in0=cc[:], scalar1=1.0)
                    innov = pe.tile([P, H], fp32, name="innov")
                    nc.vector.tensor_sub(out=innov[:], in0=act32[:], in1=p0[:])
                    if debug:
                        nc.sync.dma_start(dbg["dbg_act"][tsl, :], act32[:])
                        nc.sync.dma_start(dbg["dbg_cc"][tsl, :], cc[:])
                    for j in range(S):
                        if j == 0:
                            pj = p0[:]
                        else:
                            pjt = pe.tile([P, H], fp32, name="pjt")
                            nc.sync.dma_start(pjt[:], pred_d[j, tsl, :])
                            pj = pjt[:]
                        cj = pe.tile([P, H], fp32, name="cj")
                        nc.vector.tensor_scalar_mul(out=cj[:], in0=innov[:],
                                                    scalar1=cc[:, j:j + 1])
                        nc.vector.tensor_add(out=cj[:], in0=cj[:], in1=pj)
                        if j == 0:
                            nc.vector.tensor_mul(out=cj[:], in0=cj[:],
                                                 in1=cosc_sb[:])
                        nc.sync.dma_start(out_d[j, tsl, :], cj[:])

    nc.compile()
    return nc


def _prep_in_maps(inputs):
    f32 = np.float32
    hs = np.asarray(inputs["hidden_states"], f32)        # [4,2,1024,2048]
    cos = np.asarray(inputs["cos"], f32).reshape(NTOK, HD)
    sin = np.asarray(inputs["sin"], f32).reshape(NTOK, HD)
    sin_eff = sin.copy()
    sin_eff[:, :HD // 2] = -sin_eff[:, :HD // 2]
    wq = np.asarray(inputs["wq"], f32)
    wk = np.asarray(inputs["wk"], f32)
    wv = np.asarray(inputs["wv"], f32)
    wo = np.asarray(inputs["wo"], f32)
    gw = np.asarray(inputs["gate_w"], f32)
    uw = np.asarray(inputs["up_w"], f32)
    dw = np.asarray(inputs["down_w"], f32)
    trimask = np.triu(np.full((P, P), -1e30, f32), k=1)

    def pm(X):
        # [R, C] -> partition-major [128, R//128, C] so device DMA is contiguous
        R, C = X.shape
        return np.ascontiguousarray(
            X.reshape(R // P, P, C).transpose(1, 0, 2)).astype(BF)

    woT_full = pm(wo.T)
    llT = pm(np.asarray(inputs["laurel_left_w"], f32).T)
    lrT = np.ascontiguousarray(np.asarray(inputs["laurel_right_w"], f32).T).astype(BF)
    routerT = pm(np.asarray(inputs["router_w"], f32).T)
    predT = np.ascontiguousarray(np.asarray(inputs["pred_coef_w"], f32).T).astype(BF)
    corrT = np.ascontiguousarray(np.asarray(inputs["corr_coef_w"], f32).T).astype(BF)
    coscale = np.broadcast_to(
        np.asarray(inputs["correct_output_scale"], f32), (P, H)).copy()
    in_maps = []
    for c in range(NCORE):
        b, t0 = c // 4, (c % 4) * TOK
        g = c // 4
        wqh = wq[c * HD:(c + 1) * HD].T                   # [H, 256]
        wkh = wk[g * HD:(g + 1) * HD].T
        gT = gw[c * FFC:(c + 1) * FFC].T          # [H, FFC]
        gT4 = np.ascontiguousarray(
            gT.reshape(HC, P, 8, P).transpose(1, 2, 0, 3)).astype(BF)
        uT = uw[c * FFC:(c + 1) * FFC].T
        uT4 = np.ascontiguousarray(
            uT.reshape(HC, P, 8, P).transpose(1, 2, 0, 3)).astype(BF)
        in_maps.append({
            "h_in": np.ascontiguousarray(hs[:, b, t0:t0 + TOK, :]),
            "cosq": cos, "sinq": sin_eff, "trimask": trimask,
            "wqkT": pm(np.concatenate([wqh, wkh], 1)),
            "wvT": pm(wv[g * HD:(g + 1) * HD].T),
            "woT": woT_full,
            "gateT": gT4, "upT": uT4,
            "downT": pm(dw[:, c * FFC:(c + 1) * FFC].T),
            "llT": llT, "lrT": lrT, "routerT": routerT, "predT": predT,
            "corrT": corrT, "coscale": coscale,
        })
    return in_maps


def kernel(**inputs):
    global _NC_CACHE
    if _NC_CACHE is None:
        _NC_CACHE = build_nc(debug=os.environ.get("G3N_DEBUG") == "1")
    nc = _NC_CACHE
    in_maps = _prep_in_maps(inputs)
    r = run_bass_kernel_spmd(nc, in_maps, core_ids=list(range(NCORE)))
    out = np.empty((S, B, T, H), np.float32)
    for c in range(NCORE):
        b, t0 = c // 4, (c % 4) * TOK
        out[:, b, t0:t0 + TOK, :] = r.results[c]["out"]
    return out

